# revision 24
# baseline (speedup 1.0000x reference)
"""Quantum angle-encoder state-vector kernel for Trainium2 (8 NeuronCores).

For each batch row b and qubit q the gate rz*ry applied to |0> contributes a
2-vector col0 = cos(ry/2)e^{-i rz/2}, col1 = sin(ry/2)e^{+i rz/2}; the output
state is the Kronecker product over 16 qubits (qubit 0 = MSB), [B, 2^16] c64.

Per core (32 batch rows, pure data parallel over 8 cores) the HBM write of the
[32, 65536] c64 output (16.8 MB) is the roofline (~44us at ~380 GB/s), so the
design minimizes the serial head before the output stream starts:

  * v = v_hi (x) v_lo with v_hi/v_lo the 8-qubit half-products (length 256),
    built in POLAR form stacked on 64 partitions (rows 0:32 hi, 32:64 lo).
  * Phases are additive -> ONE K=32 bf16 selection matmul (split-bf16 h/l
    terms keep ~1e-4 absolute accuracy) computes all 256 phase sums per row.
  * Magnitudes multiply -> 7-step doubling chain on the VECTOR engine using
    scalar_tensor_tensor with a per-partition column operand (+ a zeros
    tile); the chain interleaves with the range-reduction ops so it hides
    under the PE matmul. The ONLY activation function used anywhere is Sin:
    one table load, prefetched by a dummy activation while the input DMAs
    are in flight (every ACT function switch costs a 1.5us table reload).
  * Range reduction in "turns": n = round(theta/2pi (+1/4 for the cos
    half)) via the DVE f32->i32 rounding cast; d = theta/2pi - n fused in
    one scalar_tensor_tensor; cos/sin = Sin(2pi d + pi/2) / Sin(2pi d)
    (LUT domain is [-pi, pi]: both arguments stay inside).
  * The 256x256 outer product per row is a K=2 bf16 matmul (tolerance is
    2e-2; plain bf16 factors give ~2e-3): rhs columns pre-interleaved so
    PSUM comes out in complex64 memory order. Two matmuls per batch row
    with STRIDE-2 lhsT views (even/odd output indices i=2p / i=2p+1), so
    the two PSUM tiles copied side by side into one [128,1024] SBUF tile
    (Vector/Scalar alternate) form 4KB-contiguous HBM lines; ONE plain-2D
    512KB DMA per row on the Sync HWDGE ring streams at ~419 GB/s; the
    last row splits across both rings to shrink the drain tail.

Notes for this toolchain: walrus here encodes at most ONE semaphore wait per
instruction -- _legalize_single_wait() hoists extra Tile-emitted waits into
standalone EventSemaphore instructions. Matmul operands must sit at SBUF
partition base 0/32/64, hence the LH/RH flattening DMAs. Output per core
[32,128,1024] f32 == [32, 65536] complex64 (viewed on host).
"""

import numpy as np
import ml_dtypes

import concourse.bass as bass
import concourse.mybir as mybir
import concourse.tile as tile
from concourse.bass_utils import run_bass_kernel_spmd

N_CORES = 8
B, Q = 256, 16
BC = B // N_CORES  # batch rows per core
HQ = Q // 2  # qubits per half
HL = 1 << HQ  # 256: length of each half-product
P2 = 2 * BC  # 64: both halves stacked on partitions
F32 = mybir.dt.float32
BF16 = mybir.dt.bfloat16
I32 = mybir.dt.int32
PI = float(np.pi)
PI_HALF = float(np.pi / 2)
TWO_PI = float(2.0 * np.pi)
INV2PI = float(1.0 / (2.0 * np.pi))

_AF = mybir.ActivationFunctionType
_OP = mybir.AluOpType


def _legalize_single_wait(nc):
    """This walrus build encodes at most one semaphore wait per instruction
    ("Too many sync wait commands" otherwise). Hoist extra waits into
    standalone EventSemaphore instructions placed immediately before — a
    sequencer-level wait gates everything after it on the same engine, so
    semantics are preserved (slightly stronger ordering)."""
    cnt = 0
    for fn in nc.m.functions:
        for blk in fn.blocks:
            out = []
            for ins in blk.instructions:
                si = ins.sync_info
                if si is not None and si.on_wait is not None and len(si.on_wait) > 1:
                    waits = list(si.on_wait)
                    for w in waits[:-1]:
                        cnt += 1
                        ev = mybir.InstEventSemaphore(
                            name=f"{ins.name}-presync-{cnt}", ins=[], outs=[]
                        )
                        ev.engine = ins.engine
                        ev.sync_info = mybir.SyncInfo(on_wait=[w], on_update=[])
                        out.append(ev)
                    ins.sync_info = mybir.SyncInfo(
                        on_wait=[waits[-1]], on_update=list(si.on_update)
                    )
                out.append(ins)
            try:
                blk.instructions = out
            except Exception:
                blk.instructions[:] = out
    return cnt


def _sel_matrix():
    """[32, 256] phase-selection matrix, rows = (h/l group 2) x (t 2) x
    (q 8). Qubit column 0 = MSB of the half-index."""
    sel = np.zeros((32, HL), dtype=np.float32)
    j = np.arange(HL)
    for g in range(2):
        for t in range(2):
            for q in range(HQ):
                bits = (j >> (HQ - 1 - q)) & 1
                sel[g * 16 + t * HQ + q, :] = (bits == t).astype(np.float32)
    return sel.astype(ml_dtypes.bfloat16)


def build_bass():
    nc = bass.Bass()
    ry_d = nc.dram_tensor("ry", [BC, Q], F32, kind="ExternalInput")
    rz_d = nc.dram_tensor("rz", [BC, Q], F32, kind="ExternalInput")
    # Partition p of a row's staging tile holds output indices i=2p (cols
    # 0:512) and i=2p+1 (cols 512:1024) -> every output DMA line is 4KB
    # contiguous in HBM ([bi, p] block = i 2p..2p+1, all j).
    out_d = nc.dram_tensor("out", [BC, 128, 1024], F32, kind="ExternalOutput")

    ident_d = nc.inline_tensor(
        np.eye(P2).astype(ml_dtypes.bfloat16), name="ident_const"
    )
    sel_d = nc.inline_tensor(_sel_matrix(), name="sel_const")

    with tile.TileContext(nc) as tc:
        with (
            tc.tile_pool(name="io", bufs=1) as io,
            tc.tile_pool(name="stage", bufs=8) as stage,
            tc.tile_pool(name="psum", bufs=8, space="PSUM") as psum,
        ):
            # ---- inputs + constants (Sync/Scalar HWDGE rings in parallel) --
            # Stacked angle layout [64, 8]: rows 0..31 = qubits 0..7, rows
            # 32..63 = qubits 8..15 (same batch rows).
            sry = io.tile([P2, HQ], F32, tag="sry")
            srz = io.tile([P2, HQ], F32, tag="srz")
            nc.sync.dma_start(sry[0:BC, :], ry_d[:, 0:HQ])
            nc.scalar.dma_start(sry[BC:P2, :], ry_d[:, HQ:Q])
            nc.sync.dma_start(srz[0:BC, :], rz_d[:, 0:HQ])
            nc.scalar.dma_start(srz[BC:P2, :], rz_d[:, HQ:Q])
            ident = io.tile([P2, P2], BF16, tag="ident")
            nc.sync.dma_start(ident[:], ident_d[:])
            sel = io.tile([32, HL], BF16, tag="sel")
            nc.sync.dma_start(sel[:], sel_d[:])

            # Sin-table prefetch: the only ACT function used anywhere; the
            # ~1.5us PWP load overlaps the input-DMA flight.
            warm = io.tile([P2, 1], F32, tag="warm")
            nc.gpsimd.memset(warm[:], 0.25)
            pih = io.tile([P2, 1], F32, tag="pih")
            nc.gpsimd.memset(pih[:], PI_HALF)
            zeros = io.tile([P2, 128], F32, tag="zeros")
            nc.gpsimd.memset(zeros[:], 0.0)
            wo = io.tile([P2, 1], F32, tag="wo")
            nc.scalar.activation(wo[:], warm[:], _AF.Sin)

            # ---- c = cos(ry/2), s = sin(ry/2) via ONE Sin activation ------
            xin = io.tile([P2, 16], F32, tag="xin")
            nc.vector.tensor_scalar(
                xin[:, 0:HQ], sry[:], 0.5, PI_HALF, op0=_OP.mult, op1=_OP.add
            )
            nc.vector.tensor_scalar(xin[:, HQ:16], sry[:], 0.5, None, op0=_OP.mult)
            cs8 = io.tile([P2, 16], F32, tag="cs8")
            nc.scalar.activation(cs8[:], xin[:], _AF.Sin)

            # ---- phases: phi0 = pi[c<0] - rz/2, phi1 = pi[s<0] + rz/2 -----
            a8 = io.tile([P2, 16], F32, tag="a8")  # |c| cols 0:8, |s| 8:16
            nc.vector.scalar_tensor_tensor(
                a8[:], cs8[:], -1.0, cs8[:], op0=_OP.mult, op1=_OP.max
            )
            nega = io.tile([P2, 16], F32, tag="nega")
            nc.vector.tensor_scalar(nega[:], a8[:], -1.0, None, op0=_OP.mult)
            hrz = io.tile([P2, HQ], F32, tag="hrz")
            nc.vector.tensor_scalar_mul(hrz[:], srz[:], 0.5)
            mk = io.tile([P2, 16], F32, tag="mk")
            nc.vector.tensor_scalar(mk[:], cs8[:], 0.0, None, op0=_OP.is_lt)
            phim = io.tile([P2, 16], F32, tag="phim")
            nc.vector.scalar_tensor_tensor(
                phim[:, 0:HQ], mk[:, 0:HQ], PI, hrz[:], op0=_OP.mult, op1=_OP.subtract
            )
            nc.vector.scalar_tensor_tensor(
                phim[:, HQ:16], mk[:, HQ:16], PI, hrz[:], op0=_OP.mult, op1=_OP.add
            )
            # split2-bf16: h + l covers ~17 mantissa bits (phases up to ~5.6)
            phim2 = io.tile([P2, 32], BF16, tag="phim2")
            nc.vector.tensor_copy(phim2[:, 0:16], phim[:])
            resid = io.tile([P2, 16], F32, tag="resid")
            nc.vector.tensor_sub(resid[:], phim[:], phim2[:, 0:16])
            nc.vector.tensor_copy(phim2[:, 16:32], resid[:])

            # ---- one transpose + one K=32 selection matmul ----------------
            tp = psum.tile([32, P2], BF16, tag="tp", bufs=1)
            nc.tensor.transpose(tp[:], phim2[:], ident[:])
            vals = io.tile([32, P2], BF16, tag="vals")
            nc.vector.tensor_copy(vals[:], tp[:])
            th = psum.tile([P2, HL], F32, tag="th", bufs=1)
            nc.tensor.matmul(th[:], vals[:], sel[:], start=True, stop=True)

            # ---- magnitude chain on DVE (per-partition column operand) ----
            # Interleaved with the reduction ops below so it runs while the
            # PE does the transpose+matmul.
            mA = io.tile([P2, HL], F32, tag="mA")
            mB = io.tile([P2, HL], F32, tag="mB")
            nc.vector.tensor_copy(mA[:, 0:1], a8[:, HQ - 1 : HQ])
            nc.vector.tensor_copy(mA[:, 1:2], a8[:, 15:16])
            chain = []  # (dst, src, col, L)
            cur, nxt = mA, mB
            L = 2
            for q in range(HQ - 2, 0, -1):  # stop at L=128: last level fuses
                for t in range(2):
                    chain.append((nxt, cur, t * HQ + q, t * L, L))
                cur, nxt = nxt, cur
                L *= 2
            m128 = cur  # [64, 0:128] f32 after the chain runs

            def emit_chain(steps):
                for dst, src, col, off, ln in steps:
                    nc.vector.scalar_tensor_tensor(
                        dst[:, off : off + ln], src[:, 0:ln],
                        a8[:, col : col + 1], zeros[:, 0:ln],
                        op0=_OP.mult, op1=_OP.add,
                    )

            emit_chain(chain[:8])  # lengths 2..16: runs under transpose+mm

            # ---- range-reduced cos/sin ------------------------------------
            ni = io.tile([P2, 512], I32, tag="ni")
            nc.vector.tensor_scalar(
                ni[:, 0:HL], th[:, 0:HL], INV2PI, 0.25, op0=_OP.mult, op1=_OP.add
            )
            nc.vector.tensor_scalar(
                ni[:, HL : 2 * HL], th[:, 0:HL], INV2PI, None, op0=_OP.mult
            )
            emit_chain(chain[8:12])  # lengths 32..64
            # per-half casts/diffs so the cos activation can start before the
            # sin half's chain finishes
            nf = io.tile([P2, 512], F32, tag="nf")
            nc.vector.tensor_copy(nf[:, 0:HL], ni[:, 0:HL])
            dd = io.tile([P2, 512], F32, tag="dd")
            nc.vector.scalar_tensor_tensor(
                dd[:, 0:HL], th[:, 0:HL], INV2PI, nf[:, 0:HL],
                op0=_OP.mult, op1=_OP.subtract,
            )
            nc.vector.tensor_copy(nf[:, HL : 2 * HL], ni[:, HL : 2 * HL])
            nc.vector.scalar_tensor_tensor(
                dd[:, HL : 2 * HL], th[:, 0:HL], INV2PI, nf[:, HL : 2 * HL],
                op0=_OP.mult, op1=_OP.subtract,
            )
            cs = io.tile([P2, 512], F32, tag="cs")
            # cols 0:256 = cos(theta) = sin(2pi d_c + pi/2), 256:512 = sin
            nc.scalar.activation(
                cs[:, 0:HL], dd[:, 0:HL], _AF.Sin, bias=pih[:], scale=TWO_PI
            )
            nc.scalar.activation(
                cs[:, HL : 2 * HL], dd[:, HL : 2 * HL], _AF.Sin, scale=TWO_PI
            )

            # ---- bf16 factor tiles ----------------------------------------
            # The chain's final doubling level is FUSED into these: each
            # fused op computes (trig_slice * A_col) * m128 in one
            # scalar_tensor_tensor; the nvi variant uses the negated column
            # so PT2's source does not depend on vi.
            vr = io.tile([P2, HL], BF16, tag="vr")
            vi = io.tile([P2, HL], BF16, tag="vi")
            nvi = io.tile([P2, HL], BF16, tag="nvi")
            m128v = m128[:, 0:128]
            for t in range(2):
                sl = slice(t * 128, (t + 1) * 128)
                acol = a8[:, t * HQ : t * HQ + 1]
                nc.vector.scalar_tensor_tensor(
                    vr[:, sl], cs[:, sl], acol, m128v, op0=_OP.mult, op1=_OP.mult
                )
            for t in range(2):
                sl = slice(t * 128, (t + 1) * 128)
                nc.vector.scalar_tensor_tensor(
                    vi[:, sl], cs[:, HL + t * 128 : HL + (t + 1) * 128],
                    a8[:, t * HQ : t * HQ + 1], m128v,
                    op0=_OP.mult, op1=_OP.mult,
                )
            for t in range(2):
                nc.vector.scalar_tensor_tensor(
                    nvi[BC:P2, t * 128 : (t + 1) * 128],
                    cs[BC:P2, HL + t * 128 : HL + (t + 1) * 128],
                    nega[BC:P2, t * HQ : t * HQ + 1], m128[BC:P2, 0:128],
                    op0=_OP.mult, op1=_OP.mult,
                )
            # lo half (partitions 32:64): interleaved rhs patterns.
            #   PT1 = interleave(lr, ll), PT2 = interleave(-ll, lr)
            # Strided bf16 copies are cheap (~300ns). RH0/RH1 staging DMAs
            # gate the main loop (~3us completion latency).
            pt1 = io.tile([P2, 2 * HL], BF16, tag="pt1")
            v1 = pt1[BC:P2, :].rearrange("p (j t) -> p j t", t=2)
            nc.vector.tensor_copy(v1[:, :, 0], vr[BC:P2, :])
            nc.vector.tensor_copy(v1[:, :, 1], vi[BC:P2, :])
            pt2 = io.tile([P2, 2 * HL], BF16, tag="pt2")
            v2 = pt2[BC:P2, :].rearrange("p (j t) -> p j t", t=2)
            nc.vector.tensor_copy(v2[:, :, 0], nvi[BC:P2, :])
            nc.gpsimd.tensor_copy(v2[:, :, 1], vr[BC:P2, :])

            # ---- flatten to matmul operands (4 SBUF->SBUF DMAs) ------------
            rh = io.tile([2, BC * 2 * HL], BF16, tag="rh")
            nc.sync.dma_start(rh[0:1, :], pt1[BC:P2, :])
            nc.scalar.dma_start(rh[1:2, :], pt2[BC:P2, :])
            lh = io.tile([2, BC * HL], BF16, tag="lh")
            nc.sync.dma_start(lh[0:1, :], vr[0:BC, :])
            nc.scalar.dma_start(lh[1:2, :], vi[0:BC, :])

            # ---- main loop: 2 K=2 matmuls + 2 copies + 1 DMA per row ------
            for bi in range(BC):
                rhs = rh[:, bi * 2 * HL : (bi + 1) * 2 * HL]
                # stride-2 views: even hi values feed partitions p -> i=2p,
                # odd -> i=2p+1
                lhv = lh[:, bi * HL : (bi + 1) * HL].rearrange(
                    "k (i e) -> k i e", e=2
                )
                acc_e = psum.tile([128, 512], F32, tag="acc", bufs=6)
                nc.tensor.matmul(acc_e[:], lhv[:, :, 0], rhs, start=True, stop=True)
                acc_o = psum.tile([128, 512], F32, tag="acc", bufs=6)
                nc.tensor.matmul(acc_o[:], lhv[:, :, 1], rhs, start=True, stop=True)
                ot = stage.tile([128, 1024], F32, tag="ot")
                if bi % 2 == 0:
                    nc.vector.tensor_copy(ot[:, 0:512], acc_e[:])
                    nc.scalar.copy(ot[:, 512:1024], acc_o[:])
                else:
                    nc.scalar.copy(ot[:, 0:512], acc_e[:])
                    nc.vector.tensor_copy(ot[:, 512:1024], acc_o[:])
                if 0 < bi < BC - 1:
                    nc.sync.dma_start(out_d[bi], ot[:])
                else:
                    # split the first/last rows across both rings: earlier
                    # first bytes, shorter drain tail
                    nc.sync.dma_start(out_d[bi, :, 0:512], ot[:, 0:512])
                    nc.scalar.dma_start(out_d[bi, :, 512:1024], ot[:, 512:1024])
    _legalize_single_wait(nc)
    return nc


_nc_cache = None


def _get_nc():
    global _nc_cache
    if _nc_cache is None:
        _nc_cache = build_bass()
    return _nc_cache


def run(ry_angles, rz_angles, trace=False):
    """Shard over 8 cores, run, gather. Returns (out [B, 2**Q] c64, results)."""
    ry = np.ascontiguousarray(np.asarray(ry_angles, dtype=np.float32))
    rz = np.ascontiguousarray(np.asarray(rz_angles, dtype=np.float32))
    assert ry.shape == (B, Q) and rz.shape == (B, Q)
    nc = _get_nc()
    in_maps = [
        {
            "ry": np.ascontiguousarray(ry[k * BC : (k + 1) * BC]),
            "rz": np.ascontiguousarray(rz[k * BC : (k + 1) * BC]),
        }
        for k in range(N_CORES)
    ]
    res = run_bass_kernel_spmd(nc, in_maps, list(range(N_CORES)), trace=trace)
    parts = [
        np.ascontiguousarray(r["out"]).reshape(BC, 2 * (1 << Q)).view(np.complex64)
        for r in res.results
    ]
    return np.concatenate(parts, axis=0), res


def kernel(ry_angles, rz_angles):
    out, _ = run(ry_angles, rz_angles, trace=False)
    return out


# revision 25
# speedup vs baseline: 1.1164x; 1.1164x over previous
"""Quantum angle-encoder state-vector kernel for Trainium2 (8 NeuronCores).

For each batch row b and qubit q the gate rz*ry applied to |0> contributes a
2-vector col0 = cos(ry/2)e^{-i rz/2}, col1 = sin(ry/2)e^{+i rz/2}; the output
state is the Kronecker product over 16 qubits (qubit 0 = MSB), [B, 2^16] c64.

Per core (32 batch rows, pure data parallel over 8 cores) the HBM write of the
[32, 65536] c64 output (16.8 MB) is the roofline (~44us at ~380 GB/s), so the
design minimizes the serial head before the output stream starts:

  * v = v_hi (x) v_lo with v_hi/v_lo the 8-qubit half-products (length 256),
    built in POLAR form stacked on 64 partitions (rows 0:32 hi, 32:64 lo).
  * Phases are additive -> ONE K=32 bf16 selection matmul (split-bf16 h/l
    terms keep ~1e-4 absolute accuracy) computes all 256 phase sums per row.
  * Magnitudes multiply -> 7-step doubling chain on the VECTOR engine using
    scalar_tensor_tensor with a per-partition column operand (+ a zeros
    tile); the chain interleaves with the range-reduction ops so it hides
    under the PE matmul. The ONLY activation function used anywhere is Sin:
    one table load, prefetched by a dummy activation while the input DMAs
    are in flight (every ACT function switch costs a 1.5us table reload).
  * Range reduction in "turns": n = round(theta/2pi (+1/4 for the cos
    half)) via the DVE f32->i32 rounding cast; d = theta/2pi - n fused in
    one scalar_tensor_tensor; cos/sin = Sin(2pi d + pi/2) / Sin(2pi d)
    (LUT domain is [-pi, pi]: both arguments stay inside).
  * The 256x256 outer product per row is a K=2 bf16 matmul (tolerance is
    2e-2; plain bf16 factors give ~2e-3): rhs columns pre-interleaved so
    PSUM comes out in complex64 memory order. Two matmuls per batch row
    with STRIDE-2 lhsT views (even/odd output indices i=2p / i=2p+1), so
    the two PSUM tiles copied side by side into one [128,1024] SBUF tile
    (Vector/Scalar alternate) form 4KB-contiguous HBM lines; ONE plain-2D
    512KB DMA per row on the Sync HWDGE ring streams at ~419 GB/s; the
    last row splits across both rings to shrink the drain tail.

Notes for this toolchain: walrus here encodes at most ONE semaphore wait per
instruction -- _legalize_single_wait() hoists extra Tile-emitted waits into
standalone EventSemaphore instructions. Matmul operands must sit at SBUF
partition base 0/32/64, hence the LH/RH flattening DMAs. Output per core
[32,128,1024] f32 == [32, 65536] complex64 (viewed on host).
"""

import numpy as np
import ml_dtypes

import concourse.bass as bass
import concourse.mybir as mybir
import concourse.tile as tile
from concourse.bass_utils import run_bass_kernel_spmd

N_CORES = 8
B, Q = 256, 16
BC = B // N_CORES  # batch rows per core
HQ = Q // 2  # qubits per half
HL = 1 << HQ  # 256: length of each half-product
P2 = 2 * BC  # 64: both halves stacked on partitions
F32 = mybir.dt.float32
BF16 = mybir.dt.bfloat16
I32 = mybir.dt.int32
PI = float(np.pi)
PI_HALF = float(np.pi / 2)
TWO_PI = float(2.0 * np.pi)
INV2PI = float(1.0 / (2.0 * np.pi))

_AF = mybir.ActivationFunctionType
_OP = mybir.AluOpType


def _legalize_single_wait(nc):
    """This walrus build encodes at most one semaphore wait per instruction
    ("Too many sync wait commands" otherwise). Hoist extra waits into
    standalone EventSemaphore instructions placed immediately before — a
    sequencer-level wait gates everything after it on the same engine, so
    semantics are preserved (slightly stronger ordering)."""
    cnt = 0
    for fn in nc.m.functions:
        for blk in fn.blocks:
            out = []
            for ins in blk.instructions:
                si = ins.sync_info
                if si is not None and si.on_wait is not None and len(si.on_wait) > 1:
                    waits = list(si.on_wait)
                    for w in waits[:-1]:
                        cnt += 1
                        ev = mybir.InstEventSemaphore(
                            name=f"{ins.name}-presync-{cnt}", ins=[], outs=[]
                        )
                        ev.engine = ins.engine
                        ev.sync_info = mybir.SyncInfo(on_wait=[w], on_update=[])
                        out.append(ev)
                    ins.sync_info = mybir.SyncInfo(
                        on_wait=[waits[-1]], on_update=list(si.on_update)
                    )
                out.append(ins)
            try:
                blk.instructions = out
            except Exception:
                blk.instructions[:] = out
    return cnt


def _sel_matrix():
    """[32, 256] phase-selection matrix, rows = (h/l group 2) x (t 2) x
    (q 8). Qubit column 0 = MSB of the half-index."""
    sel = np.zeros((32, HL), dtype=np.float32)
    j = np.arange(HL)
    for g in range(2):
        for t in range(2):
            for q in range(HQ):
                bits = (j >> (HQ - 1 - q)) & 1
                sel[g * 16 + t * HQ + q, :] = (bits == t).astype(np.float32)
    return sel.astype(ml_dtypes.bfloat16)


def build_bass():
    nc = bass.Bass()
    ry_d = nc.dram_tensor("ry", [BC, Q], F32, kind="ExternalInput")
    rz_d = nc.dram_tensor("rz", [BC, Q], F32, kind="ExternalInput")
    # Partition p of a row's staging tile holds output indices i=2p (cols
    # 0:512) and i=2p+1 (cols 512:1024) -> every output DMA line is 4KB
    # contiguous in HBM ([bi, p] block = i 2p..2p+1, all j).
    out_d = nc.dram_tensor("out", [BC, 128, 1024], F32, kind="ExternalOutput")

    ident_d = nc.inline_tensor(
        np.eye(P2).astype(ml_dtypes.bfloat16), name="ident_const"
    )
    sel_d = nc.inline_tensor(_sel_matrix(), name="sel_const")

    with tile.TileContext(nc) as tc:
        with (
            tc.tile_pool(name="io", bufs=1) as io,
            tc.tile_pool(name="stage", bufs=8) as stage,
            tc.tile_pool(name="psum", bufs=8, space="PSUM") as psum,
        ):
            # ---- inputs + constants (Sync/Scalar HWDGE rings in parallel) --
            # Stacked angle layout [64, 8]: rows 0..31 = qubits 0..7, rows
            # 32..63 = qubits 8..15 (same batch rows).
            sry = io.tile([P2, HQ], F32, tag="sry")
            srz = io.tile([P2, HQ], F32, tag="srz")
            nc.sync.dma_start(sry[0:BC, :], ry_d[:, 0:HQ])
            nc.scalar.dma_start(sry[BC:P2, :], ry_d[:, HQ:Q])
            nc.sync.dma_start(srz[0:BC, :], rz_d[:, 0:HQ])
            nc.scalar.dma_start(srz[BC:P2, :], rz_d[:, HQ:Q])
            ident = io.tile([P2, P2], BF16, tag="ident")
            nc.sync.dma_start(ident[:], ident_d[:])
            sel = io.tile([32, HL], BF16, tag="sel")
            nc.sync.dma_start(sel[:], sel_d[:])

            # Sin-table prefetch: the only ACT function used anywhere; the
            # ~1.5us PWP load overlaps the input-DMA flight.
            warm = io.tile([P2, 1], F32, tag="warm")
            nc.gpsimd.memset(warm[:], 0.25)
            pih = io.tile([P2, 1], F32, tag="pih")
            nc.gpsimd.memset(pih[:], PI_HALF)
            zeros = io.tile([P2, 128], F32, tag="zeros")
            nc.gpsimd.memset(zeros[:], 0.0)
            wo = io.tile([P2, 1], F32, tag="wo")
            nc.scalar.activation(wo[:], warm[:], _AF.Sin)

            # ---- c = cos(ry/2), s = sin(ry/2) via ONE Sin activation ------
            xin = io.tile([P2, 16], F32, tag="xin")
            nc.vector.tensor_scalar(
                xin[:, 0:HQ], sry[:], 0.5, PI_HALF, op0=_OP.mult, op1=_OP.add
            )
            nc.vector.tensor_scalar(xin[:, HQ:16], sry[:], 0.5, None, op0=_OP.mult)
            cs8 = io.tile([P2, 16], F32, tag="cs8")
            nc.scalar.activation(cs8[:], xin[:], _AF.Sin)

            # ---- phases: phi0 = pi[c<0] - rz/2, phi1 = pi[s<0] + rz/2 -----
            a8 = io.tile([P2, 16], F32, tag="a8")  # |c| cols 0:8, |s| 8:16
            nc.vector.scalar_tensor_tensor(
                a8[:], cs8[:], -1.0, cs8[:], op0=_OP.mult, op1=_OP.max
            )
            hrz = io.tile([P2, HQ], F32, tag="hrz")
            nc.vector.tensor_scalar_mul(hrz[:], srz[:], 0.5)
            mk = io.tile([P2, 16], F32, tag="mk")
            nc.vector.tensor_scalar(mk[:], cs8[:], 0.0, None, op0=_OP.is_lt)
            phim = io.tile([P2, 16], F32, tag="phim")
            nc.vector.scalar_tensor_tensor(
                phim[:, 0:HQ], mk[:, 0:HQ], PI, hrz[:], op0=_OP.mult, op1=_OP.subtract
            )
            nc.vector.scalar_tensor_tensor(
                phim[:, HQ:16], mk[:, HQ:16], PI, hrz[:], op0=_OP.mult, op1=_OP.add
            )
            # split2-bf16: h + l covers ~17 mantissa bits (phases up to ~5.6)
            phim2 = io.tile([P2, 32], BF16, tag="phim2")
            nc.vector.tensor_copy(phim2[:, 0:16], phim[:])
            resid = io.tile([P2, 16], F32, tag="resid")
            nc.vector.tensor_sub(resid[:], phim[:], phim2[:, 0:16])
            nc.vector.tensor_copy(phim2[:, 16:32], resid[:])

            # ---- one transpose + one K=32 selection matmul ----------------
            tp = psum.tile([32, P2], BF16, tag="tp", bufs=1)
            nc.tensor.transpose(tp[:], phim2[:], ident[:])
            vals = io.tile([32, P2], BF16, tag="vals")
            nc.vector.tensor_copy(vals[:], tp[:])
            th = psum.tile([P2, HL], F32, tag="th", bufs=1)
            nc.tensor.matmul(th[:], vals[:], sel[:], start=True, stop=True)

            # ---- magnitude chain on DVE (per-partition column operand) ----
            # Interleaved with the reduction ops below so it runs while the
            # PE does the transpose+matmul.
            mA = io.tile([P2, HL], F32, tag="mA")
            mB = io.tile([P2, HL], F32, tag="mB")
            nc.vector.tensor_copy(mA[:, 0:1], a8[:, HQ - 1 : HQ])
            nc.vector.tensor_copy(mA[:, 1:2], a8[:, 15:16])
            chain = []  # (dst, src, col, L)
            cur, nxt = mA, mB
            L = 2
            for q in range(HQ - 2, -1, -1):
                for t in range(2):
                    chain.append((nxt, cur, t * HQ + q, t * L, L))
                cur, nxt = nxt, cur
                L *= 2
            mag = cur  # [64, 256] f32 after the chain runs

            def emit_chain(steps):
                for dst, src, col, off, ln in steps:
                    nc.vector.scalar_tensor_tensor(
                        dst[:, off : off + ln], src[:, 0:ln],
                        a8[:, col : col + 1], zeros[:, 0:ln],
                        op0=_OP.mult, op1=_OP.add,
                    )

            emit_chain(chain[:8])  # lengths 2..16: runs under transpose+mm

            # ---- range-reduced cos/sin ------------------------------------
            ni = io.tile([P2, 512], I32, tag="ni")
            nc.vector.tensor_scalar(
                ni[:, 0:HL], th[:, 0:HL], INV2PI, 0.25, op0=_OP.mult, op1=_OP.add
            )
            nc.vector.tensor_scalar(
                ni[:, HL : 2 * HL], th[:, 0:HL], INV2PI, None, op0=_OP.mult
            )
            emit_chain(chain[8:12])  # lengths 32..64
            # per-half casts/diffs so the cos activation can start before the
            # sin half's chain finishes
            nf = io.tile([P2, 512], F32, tag="nf")
            nc.vector.tensor_copy(nf[:, 0:HL], ni[:, 0:HL])
            dd = io.tile([P2, 512], F32, tag="dd")
            nc.vector.scalar_tensor_tensor(
                dd[:, 0:HL], th[:, 0:HL], INV2PI, nf[:, 0:HL],
                op0=_OP.mult, op1=_OP.subtract,
            )
            nc.vector.tensor_copy(nf[:, HL : 2 * HL], ni[:, HL : 2 * HL])
            emit_chain(chain[12:])  # lengths 128
            nc.vector.scalar_tensor_tensor(
                dd[:, HL : 2 * HL], th[:, 0:HL], INV2PI, nf[:, HL : 2 * HL],
                op0=_OP.mult, op1=_OP.subtract,
            )
            cs = io.tile([P2, 512], F32, tag="cs")
            # cols 0:256 = cos(theta) = sin(2pi d_c + pi/2), 256:512 = sin
            nc.scalar.activation(
                cs[:, 0:HL], dd[:, 0:HL], _AF.Sin, bias=pih[:], scale=TWO_PI
            )
            nc.scalar.activation(
                cs[:, HL : 2 * HL], dd[:, HL : 2 * HL], _AF.Sin, scale=TWO_PI
            )

            # ---- bf16 factor tiles ----------------------------------------
            vr = io.tile([P2, HL], BF16, tag="vr")
            vi = io.tile([P2, HL], BF16, tag="vi")
            nc.vector.tensor_mul(vr[:], mag[:], cs[:, 0:HL])
            nc.vector.tensor_mul(vi[:], mag[:], cs[:, HL : 2 * HL])
            # lo half (partitions 32:64): interleaved rhs patterns.
            #   PT1 = interleave(lr, ll), PT2 = interleave(-ll, lr)
            # Strided bf16 copies are cheap (~300ns); the negate is done as a
            # plain contiguous op first. RH0/RH1 staging DMAs gate the main
            # loop (~3us completion latency), so their sources come first.
            pt1 = io.tile([P2, 2 * HL], BF16, tag="pt1")
            v1 = pt1[BC:P2, :].rearrange("p (j t) -> p j t", t=2)
            nc.vector.tensor_copy(v1[:, :, 0], vr[BC:P2, :])
            nc.vector.tensor_copy(v1[:, :, 1], vi[BC:P2, :])
            nvi = io.tile([P2, HL], BF16, tag="nvi")
            nc.vector.tensor_scalar(
                nvi[BC:P2, :], vi[BC:P2, :], -1.0, None, op0=_OP.mult
            )
            pt2 = io.tile([P2, 2 * HL], BF16, tag="pt2")
            v2 = pt2[BC:P2, :].rearrange("p (j t) -> p j t", t=2)
            nc.vector.tensor_copy(v2[:, :, 0], nvi[BC:P2, :])
            nc.gpsimd.tensor_copy(v2[:, :, 1], vr[BC:P2, :])

            # ---- flatten to matmul operands (4 SBUF->SBUF DMAs) ------------
            rh = io.tile([2, BC * 2 * HL], BF16, tag="rh")
            nc.sync.dma_start(rh[0:1, :], pt1[BC:P2, :])
            nc.scalar.dma_start(rh[1:2, :], pt2[BC:P2, :])
            lh = io.tile([2, BC * HL], BF16, tag="lh")
            nc.sync.dma_start(lh[0:1, :], vr[0:BC, :])
            nc.scalar.dma_start(lh[1:2, :], vi[0:BC, :])

            # ---- main loop: 2 K=2 matmuls + 2 copies + 1 DMA per row ------
            for bi in range(BC):
                rhs = rh[:, bi * 2 * HL : (bi + 1) * 2 * HL]
                # stride-2 views: even hi values feed partitions p -> i=2p,
                # odd -> i=2p+1
                lhv = lh[:, bi * HL : (bi + 1) * HL].rearrange(
                    "k (i e) -> k i e", e=2
                )
                acc_e = psum.tile([128, 512], F32, tag="acc", bufs=6)
                nc.tensor.matmul(acc_e[:], lhv[:, :, 0], rhs, start=True, stop=True)
                acc_o = psum.tile([128, 512], F32, tag="acc", bufs=6)
                nc.tensor.matmul(acc_o[:], lhv[:, :, 1], rhs, start=True, stop=True)
                ot = stage.tile([128, 1024], F32, tag="ot")
                if bi % 2 == 0:
                    nc.vector.tensor_copy(ot[:, 0:512], acc_e[:])
                    nc.scalar.copy(ot[:, 512:1024], acc_o[:])
                else:
                    nc.scalar.copy(ot[:, 0:512], acc_e[:])
                    nc.vector.tensor_copy(ot[:, 512:1024], acc_o[:])
                if 0 < bi < BC - 1:
                    nc.sync.dma_start(out_d[bi], ot[:])
                else:
                    # split the first/last rows across both rings: earlier
                    # first bytes, shorter drain tail
                    nc.sync.dma_start(out_d[bi, :, 0:512], ot[:, 0:512])
                    nc.scalar.dma_start(out_d[bi, :, 512:1024], ot[:, 512:1024])
    _legalize_single_wait(nc)
    return nc


_nc_cache = None


def _get_nc():
    global _nc_cache
    if _nc_cache is None:
        _nc_cache = build_bass()
    return _nc_cache


def run(ry_angles, rz_angles, trace=False):
    """Shard over 8 cores, run, gather. Returns (out [B, 2**Q] c64, results)."""
    ry = np.ascontiguousarray(np.asarray(ry_angles, dtype=np.float32))
    rz = np.ascontiguousarray(np.asarray(rz_angles, dtype=np.float32))
    assert ry.shape == (B, Q) and rz.shape == (B, Q)
    nc = _get_nc()
    in_maps = [
        {
            "ry": np.ascontiguousarray(ry[k * BC : (k + 1) * BC]),
            "rz": np.ascontiguousarray(rz[k * BC : (k + 1) * BC]),
        }
        for k in range(N_CORES)
    ]
    res = run_bass_kernel_spmd(nc, in_maps, list(range(N_CORES)), trace=trace)
    parts = [
        np.ascontiguousarray(r["out"]).reshape(BC, 2 * (1 << Q)).view(np.complex64)
        for r in res.results
    ]
    return np.concatenate(parts, axis=0), res


def kernel(ry_angles, rz_angles):
    out, _ = run(ry_angles, rz_angles, trace=False)
    return out
